# revision 25
# baseline (speedup 1.0000x reference)
"""Trainium2 Bass kernel for nn_CacheModel (retrieval_knn).

Computes out = log(exp(theta * (x/||x||) @ mem_keys) @ mem_vals) on 8
NeuronCores.  mem_keys is sharded column-wise and mem_vals row-wise over
the N_mem axis; each core computes its partial [1,1000] product, an
on-device AllReduce sums the partials, and each core takes the log.

Precision strategy: fp32 matmuls on trn2 lower to 2 hardware passes AND
do not register as PE activity for the HAM clock gate (PE stuck at
1.2 GHz).  Instead we ship each fp32 operand as a (hi, lo) split pair
and compute a @ b ~= [ah al]@bh + ah@bl with an M=2 stationary trick —
same PE cycles as fp32, but at the full 2.4 GHz warm clock.  vals use
bf16+bf16 (~2^-16 rel); keys use bf16 hi + *fp8e4m3* lo scaled by 16
(x-hi is pre-divided by 16 so the scales cancel inside the matmul),
cutting keys DMA bytes 4B->3B per element at ~2^-13 rel accuracy.

Self-contained: hardcodes all shapes; imports only the system-installed
concourse stack + numpy.
"""

from contextlib import ExitStack

import ml_dtypes
import numpy as np

import concourse.bass as bass
import concourse.tile as tile
from concourse import bacc, mybir

F32 = mybir.dt.float32
BF16 = mybir.dt.bfloat16
F8 = mybir.dt.float8e4
F8L = mybir.dt.float8e5
AF = mybir.ActivationFunctionType
BF16_NP = ml_dtypes.bfloat16
F8_NP = ml_dtypes.float8_e4m3
F8L_NP = ml_dtypes.float8_e5m2
F8_SCALE = 16.0  # keys-lo residual premultiplied by this; x-hi divided by it

# Problem shapes (full)
D_FEAT = 2048
N_MEM = 200000
N_CLASSES = 1000
THETA = 5.0
N_CORES = 8

# Per-core sharding: 25000 n-rows, zero-padded to 25088 = 196*128 = 49*512
N_SHARD = N_MEM // N_CORES          # 25000
WIN = 512                           # n-window width (one psum bank of f32)
N_PAD = 25088                       # 49 windows * 512
N_WINDOWS = N_PAD // WIN            # 49
CHUNKS_PER_WIN = WIN // 128         # 4
FEAT_CHUNKS = D_FEAT // 128         # 16
NC_HALF = N_CLASSES // 2            # 500 (<=512 moving-free-dim limit)


def build_kernel(
    num_devices: int = N_CORES,
    d_feat: int = D_FEAT,
    n_pad: int = N_PAD,
    n_classes: int = N_CLASSES,
    win: int = WIN,
    keys_bufs: int = 4,
    vals_bufs: int = 5,
):
    """Builds + compiles the per-core Bass program (SPMD: same program on
    every core; each core receives its own keys/vals shard)."""
    feat_chunks = d_feat // 128
    n_windows = n_pad // win
    chunks_per_win = win // 128
    nc_half = n_classes // 2
    n_chunks = n_pad // 128

    nc = bacc.Bacc(
        "TRN2",
        target_bir_lowering=False,
        debug=False,
        num_devices=num_devices,
    )

    x_d = nc.dram_tensor("x", [1, d_feat], F32, kind="ExternalInput").ap()
    # keys/vals arrive host-retiled AND hi/lo bf16-split; each window is one
    # contiguous block with contiguous per-partition runs:
    #   k{h,l}[w, p, c*win + j]  = bf16 split of keys_shard[c*128+p, w*win+j]
    #   v{h,l}[w, p, q*ncls + j] = bf16 split of vals_shard[(w*cpw+q)*128+p, j]
    kh_d = nc.dram_tensor(
        "kh", [n_windows, 128, feat_chunks * win], BF16, kind="ExternalInput"
    ).ap()
    kl_d = nc.dram_tensor(
        "kl", [n_windows, 128, feat_chunks * win], F8, kind="ExternalInput"
    ).ap()
    vh_d = nc.dram_tensor(
        "vh", [n_windows, 128, chunks_per_win * n_classes], BF16,
        kind="ExternalInput",
    ).ap()
    # vals residual in fp8e5m2: subnormals reach 2^-16 so no scaling needed;
    # pairs with the bf16 stationary (mixed non-fp32 matmul dtypes are legal).
    vl_d = nc.dram_tensor(
        "vl", [n_windows, 128, chunks_per_win * n_classes], F8L,
        kind="ExternalInput",
    ).ap()
    out_d = nc.dram_tensor("out", [1, n_classes], F32, kind="ExternalOutput").ap()

    with tile.TileContext(nc) as tc, ExitStack() as ctx:
        const = ctx.enter_context(tc.tile_pool(name="const", bufs=1))
        keys_pool = ctx.enter_context(tc.tile_pool(name="keys", bufs=keys_bufs))
        vals_pool = ctx.enter_context(tc.tile_pool(name="vals", bufs=vals_bufs))
        s_pool = ctx.enter_context(tc.tile_pool(name="s", bufs=4))
        st_pool = ctx.enter_context(tc.tile_pool(name="st", bufs=4))
        psum_s = ctx.enter_context(tc.tile_pool(name="psum_s", bufs=3, space="PSUM"))
        psum_t = ctx.enter_context(tc.tile_pool(name="psum_t", bufs=2, space="PSUM"))
        psum_p = ctx.enter_context(tc.tile_pool(name="psum_p", bufs=1, space="PSUM"))
        dram = ctx.enter_context(tc.tile_pool(name="dram", bufs=1, space="DRAM"))

        # ---- prologue: xt = x reshaped [128, feat_chunks]; scale = theta/||x||
        xt = const.tile([128, feat_chunks], F32)
        nc.sync.dma_start(out=xt[:], in_=x_d.rearrange("a (c p) -> p (a c)", p=128))

        ones = const.tile([128, 1], F32)
        nc.vector.memset(ones[:], 1.0)

        sq = const.tile([128, feat_chunks], F32)
        nc.vector.tensor_mul(sq[:], xt[:], xt[:])
        sums = const.tile([128, 1], F32)
        nc.vector.tensor_reduce(
            sums[:], sq[:], axis=mybir.AxisListType.X, op=mybir.AluOpType.add
        )
        nrm2_ps = psum_t.tile([1, 1], F32, tag="ps_t")
        nc.tensor.matmul(nrm2_ps[:], lhsT=ones[:], rhs=sums[:], start=True, stop=True)
        nrm = const.tile([1, 1], F32)
        nc.scalar.sqrt(nrm[:], nrm2_ps[:])
        inv = const.tile([1, 1], F32)
        nc.vector.reciprocal(inv[:], nrm[:])
        scale = const.tile([1, 1], F32)
        nc.vector.tensor_scalar_mul(scale[:], inv[:], THETA)
        ones_row = const.tile([1, 2], F32)
        nc.vector.memset(ones_row[:], 1.0)
        sc2_ps = psum_t.tile([2, 1], F32, tag="ps_t")
        nc.tensor.matmul(sc2_ps[:], lhsT=ones_row[:], rhs=scale[:], start=True, stop=True)
        scale2 = const.tile([2, 1], F32)
        nc.vector.tensor_copy(scale2[:], sc2_ps[:])

        # x hi/lo bf16 split, interleaved as xs[:, c, 0]=xh, xs[:, c, 1]=xl
        xh_bf = const.tile([128, feat_chunks], BF16)
        nc.vector.tensor_copy(xh_bf[:], xt[:])
        xh32 = const.tile([128, feat_chunks], F32)
        nc.vector.tensor_copy(xh32[:], xh_bf[:])
        xl32 = const.tile([128, feat_chunks], F32)
        nc.vector.tensor_sub(xl32[:], xt[:], xh32[:])
        xs = const.tile([128, feat_chunks, 2], BF16)
        nc.vector.tensor_copy(xs[:, :, 0:1], xh_bf[:].rearrange("p (c o) -> p c o", o=1))
        nc.vector.tensor_copy(xs[:, :, 1:2], xl32[:].rearrange("p (c o) -> p c o", o=1))
        # x-hi scaled down for the fp8 keys-residual pass (scales cancel in MM)
        xsc = const.tile([128, feat_chunks], F32)
        nc.vector.tensor_scalar_mul(xsc[:], xh32[:], 1.0 / F8_SCALE)
        xfp = const.tile([128, feat_chunks], F8)
        nc.vector.tensor_copy(xfp[:], xsc[:])

        # ---- persistent [2, nc_half] accumulators (row0: hi-part, row1: lo-x part)
        pp_a = psum_p.tile([2, nc_half], F32, tag="pp_a")
        pp_b = psum_p.tile([2, nc_half], F32, tag="pp_b")

        def emit_post(ps_s, vh, vl, w):
            # fused: ps_t[128,1] = scale*(row0+row1) transposed, per 128-chunk
            s2 = s_pool.tile([2, win], F32, tag="s2")
            nc.vector.tensor_copy(s2[:], ps_s[:])
            ss = st_pool.tile([128, chunks_per_win, 2], BF16)
            for q in range(chunks_per_win):
                ps_t = psum_t.tile([128, 1], F32, tag="ps_t")
                nc.tensor.matmul(
                    ps_t[:],
                    lhsT=s2[:, q * 128:(q + 1) * 128],
                    rhs=scale2[:],
                    start=True,
                    stop=True,
                )
                se = st_pool.tile([128, 1], F32, tag="se")
                nc.scalar.activation(se[:], ps_t[:], AF.Exp)
                nc.vector.tensor_copy(ss[:, q, 0:1], se[:])
                sh32 = st_pool.tile([128, 1], F32, tag="sh32")
                nc.vector.tensor_copy(sh32[:], ss[:, q, 0:1])
                sl32 = st_pool.tile([128, 1], F32, tag="sl32")
                nc.vector.tensor_sub(sl32[:], se[:], sh32[:])
                nc.vector.tensor_copy(ss[:, q, 1:2], sl32[:])
            # stage 2: pp[0,:] += sh@Vh + sh@Vl ; pp[1,:] += sl@Vh
            for q in range(chunks_per_win):
                gc = w * chunks_per_win + q
                first = gc == 0
                last = gc == n_chunks - 1
                for pp, j0 in ((pp_a, 0), (pp_b, nc_half)):
                    nc.tensor.matmul(
                        pp[:],
                        lhsT=ss[:, q, :],
                        rhs=vh[:, q, j0:j0 + nc_half],
                        start=first,
                        stop=False,
                        skip_group_check=True,
                    )
                    nc.tensor.matmul(
                        pp[0:1, :],
                        lhsT=ss[:, q, 0:1],
                        rhs=vl[:, q, j0:j0 + nc_half],
                        start=False,
                        stop=last,
                        skip_group_check=True,
                    )

        # Software-pipelined emission: window w's post-chain (rowsum/exp/
        # transpose/stage-2) is emitted AFTER window w+1's stage-1 matmuls so
        # the PE stream stays dense while DVE/ACT work on the previous window.
        pend = None
        for w in range(n_windows):
            kh = keys_pool.tile([128, feat_chunks, win], BF16, tag="keys")
            nc.sync.dma_start(
                out=kh[:], in_=kh_d[w].rearrange("p (c j) -> p c j", c=feat_chunks)
            )
            kl = keys_pool.tile([128, feat_chunks, win], F8, tag="keys_lo")
            nc.sync.dma_start(
                out=kl[:], in_=kl_d[w].rearrange("p (c j) -> p c j", c=feat_chunks)
            )
            vh = vals_pool.tile([128, chunks_per_win, n_classes], BF16, tag="vals")
            nc.sync.dma_start(
                out=vh[:], in_=vh_d[w].rearrange("p (q j) -> p q j", q=chunks_per_win)
            )
            vl = vals_pool.tile([128, chunks_per_win, n_classes], F8L, tag="vals_lo")
            nc.sync.dma_start(
                out=vl[:], in_=vl_d[w].rearrange("p (q j) -> p q j", q=chunks_per_win)
            )

            # stage 1: ps_s[0,:] = xh@Kh + xh@Kl ; ps_s[1,:] = xl@Kh
            ps_s = psum_s.tile([2, win], F32)
            for c in range(feat_chunks):
                nc.tensor.matmul(
                    ps_s[:],
                    lhsT=xs[:, c, :],
                    rhs=kh[:, c, :],
                    start=(c == 0),
                    stop=False,
                    skip_group_check=True,
                )
                nc.tensor.matmul(
                    ps_s[0:1, :],
                    lhsT=xfp[:, c:c + 1],
                    rhs=kl[:, c, :],
                    start=False,
                    stop=(c == feat_chunks - 1),
                    skip_group_check=True,
                )

            if pend is not None:
                emit_post(*pend)
            pend = (ps_s, vh, vl, w)
        emit_post(*pend)

        # ---- tail: p = row0 + row1 (copy to SBUF, K=2 ones-matmul row sum)
        p_sb = const.tile([1, n_classes], F32)
        for pp, j0 in ((pp_a, 0), (pp_b, nc_half)):
            pc = const.tile([2, nc_half], F32, tag=f"pc{j0}")
            nc.vector.tensor_copy(pc[:], pp[:])
            pr = psum_t.tile([1, nc_half], F32, tag="ps_t")
            nc.tensor.matmul(
                pr[:], lhsT=ones[0:2, 0:1], rhs=pc[:], start=True, stop=True
            )
            nc.vector.tensor_copy(p_sb[:, j0:j0 + nc_half], pr[:])

        partial = dram.tile([1, n_classes], F32)
        reduced = dram.tile([1, n_classes], F32)
        nc.gpsimd.dma_start(partial[:], p_sb[:])
        nc.gpsimd.collective_compute(
            "AllReduce",
            mybir.AluOpType.add,
            replica_groups=[list(range(num_devices))],
            ins=[partial.opt()],
            outs=[reduced.opt()],
        )
        red_sb = const.tile([1, n_classes], F32)
        nc.sync.dma_start(red_sb[:], reduced[:])
        logp = const.tile([1, n_classes], F32)
        nc.scalar.activation(logp[:], red_sb[:], AF.Ln)
        nc.sync.dma_start(out_d[:], logp[:])

    nc.compile()
    return nc


_NC_CACHE: dict = {}


def _get_nc():
    if "nc" not in _NC_CACHE:
        _NC_CACHE["nc"] = build_kernel()
    return _NC_CACHE["nc"]


def _split_hi_lo(a):
    hi = a.astype(BF16_NP)
    lo = (a - hi.astype(np.float32)).astype(BF16_NP)
    return hi, lo


def _retile_keys(keys_shard, feat_chunks=FEAT_CHUNKS, win=WIN):
    """[d_feat, n_pad] -> [n_windows, 128, feat_chunks*win] with
    out[w, p, c*win + j] = keys_shard[c*128 + p, w*win + j]."""
    d_feat, n_pad = keys_shard.shape
    n_windows = n_pad // win
    v = keys_shard.reshape(feat_chunks, 128, n_windows, win)
    return np.ascontiguousarray(v.transpose(2, 1, 0, 3)).reshape(
        n_windows, 128, feat_chunks * win
    )


def _retile_vals(vals_shard, chunks_per_win=CHUNKS_PER_WIN, win=WIN):
    """[n_pad, n_classes] -> [n_windows, 128, chunks_per_win*n_classes] with
    out[w, p, q*ncls + j] = vals_shard[(w*cpw + q)*128 + p, j]."""
    n_pad, ncls = vals_shard.shape
    n_windows = n_pad // win
    v = vals_shard.reshape(n_windows, chunks_per_win, 128, ncls)
    return np.ascontiguousarray(v.transpose(0, 2, 1, 3)).reshape(
        n_windows, 128, chunks_per_win * ncls
    )


def _shard_inputs(x, mem_keys, mem_vals):
    x = np.ascontiguousarray(np.asarray(x, dtype=np.float32))
    in_maps = []
    for i in range(N_CORES):
        lo_i, hi_i = i * N_SHARD, (i + 1) * N_SHARD
        keys_shard = np.zeros((D_FEAT, N_PAD), dtype=np.float32)
        keys_shard[:, :N_SHARD] = mem_keys[:, lo_i:hi_i]
        vals_shard = np.zeros((N_PAD, N_CLASSES), dtype=np.float32)
        vals_shard[:N_SHARD, :] = mem_vals[lo_i:hi_i, :]
        kt = _retile_keys(keys_shard)
        kh = kt.astype(BF16_NP)
        kl = ((kt - kh.astype(np.float32)) * F8_SCALE).astype(F8_NP)
        vt = _retile_vals(vals_shard)
        vh = vt.astype(BF16_NP)
        vl = (vt - vh.astype(np.float32)).astype(F8L_NP)
        in_maps.append({"x": x, "kh": kh, "kl": kl, "vh": vh, "vl": vl})
    return in_maps


def run(x, mem_keys, mem_vals, trace: bool = False):
    """Runs the SPMD kernel; returns (output [1, N_CLASSES], BassKernelResults)."""
    from concourse.bass_utils import run_bass_kernel_spmd

    nc = _get_nc()
    in_maps = _shard_inputs(x, mem_keys, mem_vals)
    res = run_bass_kernel_spmd(nc, in_maps, list(range(N_CORES)), trace=trace)
    out = np.asarray(res.results[0]["out"], dtype=np.float32).reshape(1, N_CLASSES)
    return out, res


def kernel(x, mem_keys, mem_vals):
    out, _ = run(x, mem_keys, mem_vals, trace=False)
    return out


# revision 26
# speedup vs baseline: 1.0137x; 1.0137x over previous
"""Trainium2 Bass kernel for nn_CacheModel (retrieval_knn).

Computes out = log(exp(theta * (x/||x||) @ mem_keys) @ mem_vals) on 8
NeuronCores.  mem_keys is sharded column-wise and mem_vals row-wise over
the N_mem axis; each core computes its partial [1,1000] product, an
on-device AllReduce sums the partials, and each core takes the log.

Precision strategy: fp32 matmuls on trn2 lower to 2 hardware passes AND
do not register as PE activity for the HAM clock gate (PE stuck at
1.2 GHz).  Instead we ship each fp32 operand as a (hi, lo) split pair
and compute a @ b ~= [ah al]@bh + ah@bl with an M=2 stationary trick —
same PE cycles as fp32, but at the full 2.4 GHz warm clock.  vals use
bf16+bf16 (~2^-16 rel); keys use bf16 hi + *fp8e4m3* lo scaled by 16
(x-hi is pre-divided by 16 so the scales cancel inside the matmul),
cutting keys DMA bytes 4B->3B per element at ~2^-13 rel accuracy.

Self-contained: hardcodes all shapes; imports only the system-installed
concourse stack + numpy.
"""

from contextlib import ExitStack

import ml_dtypes
import numpy as np

import concourse.bass as bass
import concourse.tile as tile
from concourse import bacc, mybir

F32 = mybir.dt.float32
BF16 = mybir.dt.bfloat16
F8 = mybir.dt.float8e4
F8L = mybir.dt.float8e5
AF = mybir.ActivationFunctionType
BF16_NP = ml_dtypes.bfloat16
F8_NP = ml_dtypes.float8_e4m3
F8L_NP = ml_dtypes.float8_e5m2
F8_SCALE = 16.0  # keys-lo residual premultiplied by this; x-hi divided by it

# Problem shapes (full)
D_FEAT = 2048
N_MEM = 200000
N_CLASSES = 1000
THETA = 5.0
N_CORES = 8

# Per-core sharding: 25000 n-rows, zero-padded to 25088 = 196*128 = 49*512
N_SHARD = N_MEM // N_CORES          # 25000
WIN = 512                           # n-window width (one psum bank of f32)
N_PAD = 25088                       # 49 windows * 512
N_WINDOWS = N_PAD // WIN            # 49
CHUNKS_PER_WIN = WIN // 128         # 4
FEAT_CHUNKS = D_FEAT // 128         # 16
NC_HALF = N_CLASSES // 2            # 500 (<=512 moving-free-dim limit)


def build_kernel(
    num_devices: int = N_CORES,
    d_feat: int = D_FEAT,
    n_pad: int = N_PAD,
    n_classes: int = N_CLASSES,
    win: int = WIN,
    keys_bufs: int = 3,
    vals_bufs: int = 7,
):
    """Builds + compiles the per-core Bass program (SPMD: same program on
    every core; each core receives its own keys/vals shard)."""
    feat_chunks = d_feat // 128
    n_windows = n_pad // win
    chunks_per_win = win // 128
    nc_half = n_classes // 2
    n_chunks = n_pad // 128

    nc = bacc.Bacc(
        "TRN2",
        target_bir_lowering=False,
        debug=False,
        num_devices=num_devices,
    )

    x_d = nc.dram_tensor("x", [1, d_feat], F32, kind="ExternalInput").ap()
    # keys/vals arrive host-retiled AND hi/lo bf16-split; each window is one
    # contiguous block with contiguous per-partition runs:
    #   k{h,l}[w, p, c*win + j]  = bf16 split of keys_shard[c*128+p, w*win+j]
    #   v{h,l}[w, p, q*ncls + j] = bf16 split of vals_shard[(w*cpw+q)*128+p, j]
    kh_d = nc.dram_tensor(
        "kh", [n_windows, 128, feat_chunks * win], BF16, kind="ExternalInput"
    ).ap()
    kl_d = nc.dram_tensor(
        "kl", [n_windows, 128, feat_chunks * win], F8, kind="ExternalInput"
    ).ap()
    vh_d = nc.dram_tensor(
        "vh", [n_windows, 128, chunks_per_win * n_classes], BF16,
        kind="ExternalInput",
    ).ap()
    # vals residual in fp8e5m2: subnormals reach 2^-16 so no scaling needed;
    # pairs with the bf16 stationary (mixed non-fp32 matmul dtypes are legal).
    vl_d = nc.dram_tensor(
        "vl", [n_windows, 128, chunks_per_win * n_classes], F8L,
        kind="ExternalInput",
    ).ap()
    out_d = nc.dram_tensor("out", [1, n_classes], F32, kind="ExternalOutput").ap()

    with tile.TileContext(nc) as tc, ExitStack() as ctx:
        const = ctx.enter_context(tc.tile_pool(name="const", bufs=1))
        keys_pool = ctx.enter_context(tc.tile_pool(name="keys", bufs=keys_bufs))
        vals_pool = ctx.enter_context(tc.tile_pool(name="vals", bufs=vals_bufs))
        s_pool = ctx.enter_context(tc.tile_pool(name="s", bufs=4))
        st_pool = ctx.enter_context(tc.tile_pool(name="st", bufs=4))
        psum_s = ctx.enter_context(tc.tile_pool(name="psum_s", bufs=3, space="PSUM"))
        psum_t = ctx.enter_context(tc.tile_pool(name="psum_t", bufs=2, space="PSUM"))
        psum_p = ctx.enter_context(tc.tile_pool(name="psum_p", bufs=1, space="PSUM"))
        dram = ctx.enter_context(tc.tile_pool(name="dram", bufs=1, space="DRAM"))

        # ---- prologue: xt = x reshaped [128, feat_chunks]; scale = theta/||x||
        xt = const.tile([128, feat_chunks], F32)
        nc.sync.dma_start(out=xt[:], in_=x_d.rearrange("a (c p) -> p (a c)", p=128))

        ones = const.tile([128, 1], F32)
        nc.vector.memset(ones[:], 1.0)

        sq = const.tile([128, feat_chunks], F32)
        nc.vector.tensor_mul(sq[:], xt[:], xt[:])
        sums = const.tile([128, 1], F32)
        nc.vector.tensor_reduce(
            sums[:], sq[:], axis=mybir.AxisListType.X, op=mybir.AluOpType.add
        )
        nrm2_ps = psum_t.tile([1, 1], F32, tag="ps_t")
        nc.tensor.matmul(nrm2_ps[:], lhsT=ones[:], rhs=sums[:], start=True, stop=True)
        nrm = const.tile([1, 1], F32)
        nc.scalar.sqrt(nrm[:], nrm2_ps[:])
        inv = const.tile([1, 1], F32)
        nc.vector.reciprocal(inv[:], nrm[:])
        scale = const.tile([1, 1], F32)
        nc.vector.tensor_scalar_mul(scale[:], inv[:], THETA)
        ones_row = const.tile([1, 2], F32)
        nc.vector.memset(ones_row[:], 1.0)
        sc2_ps = psum_t.tile([2, 1], F32, tag="ps_t")
        nc.tensor.matmul(sc2_ps[:], lhsT=ones_row[:], rhs=scale[:], start=True, stop=True)
        scale2 = const.tile([2, 1], F32)
        nc.vector.tensor_copy(scale2[:], sc2_ps[:])

        # x hi/lo bf16 split, interleaved as xs[:, c, 0]=xh, xs[:, c, 1]=xl
        xh_bf = const.tile([128, feat_chunks], BF16)
        nc.vector.tensor_copy(xh_bf[:], xt[:])
        xh32 = const.tile([128, feat_chunks], F32)
        nc.vector.tensor_copy(xh32[:], xh_bf[:])
        xl32 = const.tile([128, feat_chunks], F32)
        nc.vector.tensor_sub(xl32[:], xt[:], xh32[:])
        xs = const.tile([128, feat_chunks, 2], BF16)
        nc.vector.tensor_copy(xs[:, :, 0:1], xh_bf[:].rearrange("p (c o) -> p c o", o=1))
        nc.vector.tensor_copy(xs[:, :, 1:2], xl32[:].rearrange("p (c o) -> p c o", o=1))
        # x-hi scaled down for the fp8 keys-residual pass (scales cancel in MM)
        xsc = const.tile([128, feat_chunks], F32)
        nc.vector.tensor_scalar_mul(xsc[:], xh32[:], 1.0 / F8_SCALE)
        xfp = const.tile([128, feat_chunks], F8)
        nc.vector.tensor_copy(xfp[:], xsc[:])

        # ---- persistent [2, nc_half] accumulators (row0: hi-part, row1: lo-x part)
        pp_a = psum_p.tile([2, nc_half], F32, tag="pp_a")
        pp_b = psum_p.tile([2, nc_half], F32, tag="pp_b")

        def emit_post(ps_s, vh, vl, w):
            # fused: ps_t[128,1] = scale*(row0+row1) transposed, per 128-chunk
            s2 = s_pool.tile([2, win], F32, tag="s2")
            nc.vector.tensor_copy(s2[:], ps_s[:])
            ss = st_pool.tile([128, chunks_per_win, 2], BF16)
            for q in range(chunks_per_win):
                ps_t = psum_t.tile([128, 1], F32, tag="ps_t")
                nc.tensor.matmul(
                    ps_t[:],
                    lhsT=s2[:, q * 128:(q + 1) * 128],
                    rhs=scale2[:],
                    start=True,
                    stop=True,
                )
                se = st_pool.tile([128, 1], F32, tag="se")
                nc.scalar.activation(se[:], ps_t[:], AF.Exp)
                nc.vector.tensor_copy(ss[:, q, 0:1], se[:])
                sh32 = st_pool.tile([128, 1], F32, tag="sh32")
                nc.vector.tensor_copy(sh32[:], ss[:, q, 0:1])
                sl32 = st_pool.tile([128, 1], F32, tag="sl32")
                nc.vector.tensor_sub(sl32[:], se[:], sh32[:])
                nc.vector.tensor_copy(ss[:, q, 1:2], sl32[:])
            # stage 2: pp[0,:] += sh@Vh + sh@Vl ; pp[1,:] += sl@Vh
            for q in range(chunks_per_win):
                gc = w * chunks_per_win + q
                first = gc == 0
                last = gc == n_chunks - 1
                for pp, j0 in ((pp_a, 0), (pp_b, nc_half)):
                    nc.tensor.matmul(
                        pp[:],
                        lhsT=ss[:, q, :],
                        rhs=vh[:, q, j0:j0 + nc_half],
                        start=first,
                        stop=False,
                        skip_group_check=True,
                    )
                    nc.tensor.matmul(
                        pp[0:1, :],
                        lhsT=ss[:, q, 0:1],
                        rhs=vl[:, q, j0:j0 + nc_half],
                        start=False,
                        stop=last,
                        skip_group_check=True,
                    )

        # Software-pipelined emission: window w's post-chain (rowsum/exp/
        # transpose/stage-2) is emitted AFTER window w+1's stage-1 matmuls so
        # the PE stream stays dense while DVE/ACT work on the previous window.
        pend = None
        for w in range(n_windows):
            kh = keys_pool.tile([128, feat_chunks, win], BF16, tag="keys")
            nc.sync.dma_start(
                out=kh[:], in_=kh_d[w].rearrange("p (c j) -> p c j", c=feat_chunks)
            )
            kl = keys_pool.tile([128, feat_chunks, win], F8, tag="keys_lo")
            nc.sync.dma_start(
                out=kl[:], in_=kl_d[w].rearrange("p (c j) -> p c j", c=feat_chunks)
            )
            vh = vals_pool.tile([128, chunks_per_win, n_classes], BF16, tag="vals")
            nc.sync.dma_start(
                out=vh[:], in_=vh_d[w].rearrange("p (q j) -> p q j", q=chunks_per_win)
            )
            vl = vals_pool.tile([128, chunks_per_win, n_classes], F8L, tag="vals_lo")
            nc.sync.dma_start(
                out=vl[:], in_=vl_d[w].rearrange("p (q j) -> p q j", q=chunks_per_win)
            )

            # stage 1: ps_s[0,:] = xh@Kh + xh@Kl ; ps_s[1,:] = xl@Kh
            ps_s = psum_s.tile([2, win], F32)
            for c in range(feat_chunks):
                nc.tensor.matmul(
                    ps_s[:],
                    lhsT=xs[:, c, :],
                    rhs=kh[:, c, :],
                    start=(c == 0),
                    stop=False,
                    skip_group_check=True,
                )
                nc.tensor.matmul(
                    ps_s[0:1, :],
                    lhsT=xfp[:, c:c + 1],
                    rhs=kl[:, c, :],
                    start=False,
                    stop=(c == feat_chunks - 1),
                    skip_group_check=True,
                )

            if pend is not None:
                emit_post(*pend)
            pend = (ps_s, vh, vl, w)
        emit_post(*pend)

        # ---- tail: p = row0 + row1 (copy to SBUF, K=2 ones-matmul row sum)
        p_sb = const.tile([1, n_classes], F32)
        for pp, j0 in ((pp_a, 0), (pp_b, nc_half)):
            pc = const.tile([2, nc_half], F32, tag=f"pc{j0}")
            nc.vector.tensor_copy(pc[:], pp[:])
            pr = psum_t.tile([1, nc_half], F32, tag="ps_t")
            nc.tensor.matmul(
                pr[:], lhsT=ones[0:2, 0:1], rhs=pc[:], start=True, stop=True
            )
            nc.vector.tensor_copy(p_sb[:, j0:j0 + nc_half], pr[:])

        partial = dram.tile([1, n_classes], F32)
        reduced = dram.tile([1, n_classes], F32)
        nc.gpsimd.dma_start(partial[:], p_sb[:])
        nc.gpsimd.collective_compute(
            "AllReduce",
            mybir.AluOpType.add,
            replica_groups=[list(range(num_devices))],
            ins=[partial.opt()],
            outs=[reduced.opt()],
        )
        red_sb = const.tile([1, n_classes], F32)
        nc.sync.dma_start(red_sb[:], reduced[:])
        logp = const.tile([1, n_classes], F32)
        nc.scalar.activation(logp[:], red_sb[:], AF.Ln)
        nc.sync.dma_start(out_d[:], logp[:])

    nc.compile()
    return nc


_NC_CACHE: dict = {}


def _get_nc():
    if "nc" not in _NC_CACHE:
        _NC_CACHE["nc"] = build_kernel()
    return _NC_CACHE["nc"]


def _split_hi_lo(a):
    hi = a.astype(BF16_NP)
    lo = (a - hi.astype(np.float32)).astype(BF16_NP)
    return hi, lo


def _retile_keys(keys_shard, feat_chunks=FEAT_CHUNKS, win=WIN):
    """[d_feat, n_pad] -> [n_windows, 128, feat_chunks*win] with
    out[w, p, c*win + j] = keys_shard[c*128 + p, w*win + j]."""
    d_feat, n_pad = keys_shard.shape
    n_windows = n_pad // win
    v = keys_shard.reshape(feat_chunks, 128, n_windows, win)
    return np.ascontiguousarray(v.transpose(2, 1, 0, 3)).reshape(
        n_windows, 128, feat_chunks * win
    )


def _retile_vals(vals_shard, chunks_per_win=CHUNKS_PER_WIN, win=WIN):
    """[n_pad, n_classes] -> [n_windows, 128, chunks_per_win*n_classes] with
    out[w, p, q*ncls + j] = vals_shard[(w*cpw + q)*128 + p, j]."""
    n_pad, ncls = vals_shard.shape
    n_windows = n_pad // win
    v = vals_shard.reshape(n_windows, chunks_per_win, 128, ncls)
    return np.ascontiguousarray(v.transpose(0, 2, 1, 3)).reshape(
        n_windows, 128, chunks_per_win * ncls
    )


def _shard_inputs(x, mem_keys, mem_vals):
    x = np.ascontiguousarray(np.asarray(x, dtype=np.float32))
    in_maps = []
    for i in range(N_CORES):
        lo_i, hi_i = i * N_SHARD, (i + 1) * N_SHARD
        keys_shard = np.zeros((D_FEAT, N_PAD), dtype=np.float32)
        keys_shard[:, :N_SHARD] = mem_keys[:, lo_i:hi_i]
        vals_shard = np.zeros((N_PAD, N_CLASSES), dtype=np.float32)
        vals_shard[:N_SHARD, :] = mem_vals[lo_i:hi_i, :]
        kt = _retile_keys(keys_shard)
        kh = kt.astype(BF16_NP)
        kl = ((kt - kh.astype(np.float32)) * F8_SCALE).astype(F8_NP)
        vt = _retile_vals(vals_shard)
        vh = vt.astype(BF16_NP)
        vl = (vt - vh.astype(np.float32)).astype(F8L_NP)
        in_maps.append({"x": x, "kh": kh, "kl": kl, "vh": vh, "vl": vl})
    return in_maps


def run(x, mem_keys, mem_vals, trace: bool = False):
    """Runs the SPMD kernel; returns (output [1, N_CLASSES], BassKernelResults)."""
    from concourse.bass_utils import run_bass_kernel_spmd

    nc = _get_nc()
    in_maps = _shard_inputs(x, mem_keys, mem_vals)
    res = run_bass_kernel_spmd(nc, in_maps, list(range(N_CORES)), trace=trace)
    out = np.asarray(res.results[0]["out"], dtype=np.float32).reshape(1, N_CLASSES)
    return out, res


def kernel(x, mem_keys, mem_vals):
    out, _ = run(x, mem_keys, mem_vals, trace=False)
    return out


# revision 27
# speedup vs baseline: 1.1133x; 1.0983x over previous
"""Trainium2 Bass kernel for nn_CacheModel (retrieval_knn).

Computes out = log(exp(theta * (x/||x||) @ mem_keys) @ mem_vals) on 8
NeuronCores.  mem_keys is sharded column-wise and mem_vals row-wise over
the N_mem axis; each core computes its partial [1,1000] product, an
on-device AllReduce sums the partials, and each core takes the log.

Precision strategy: fp32 matmuls on trn2 lower to 2 hardware passes AND
do not register as PE activity for the HAM clock gate (PE stuck at
1.2 GHz).  Instead we ship each fp32 operand as a (hi, lo) split pair
and compute a @ b ~= [ah al]@bh + ah@bl with an M=2 stationary trick —
same PE cycles as fp32, but at the full 2.4 GHz warm clock.  vals use
bf16+bf16 (~2^-16 rel); keys use bf16 hi + *fp8e4m3* lo scaled by 16
(x-hi is pre-divided by 16 so the scales cancel inside the matmul),
cutting keys DMA bytes 4B->3B per element at ~2^-13 rel accuracy.

Self-contained: hardcodes all shapes; imports only the system-installed
concourse stack + numpy.
"""

from contextlib import ExitStack

import ml_dtypes
import numpy as np

import concourse.bass as bass
import concourse.tile as tile
from concourse import bacc, mybir

F32 = mybir.dt.float32
BF16 = mybir.dt.bfloat16
F8 = mybir.dt.float8e4
F8L = mybir.dt.float8e5
F16 = mybir.dt.float16
AF = mybir.ActivationFunctionType
BF16_NP = ml_dtypes.bfloat16
F8_NP = ml_dtypes.float8_e4m3
F8L_NP = ml_dtypes.float8_e5m2
F8_SCALE = 16.0  # keys-lo residual premultiplied by this; x-hi divided by it

# Problem shapes (full)
D_FEAT = 2048
N_MEM = 200000
N_CLASSES = 1000
THETA = 5.0
N_CORES = 8

# Per-core sharding: 25000 n-rows, zero-padded to 25088 = 196*128 = 49*512
N_SHARD = N_MEM // N_CORES          # 25000
WIN = 512                           # n-window width (one psum bank of f32)
N_PAD = 25088                       # 49 windows * 512
N_WINDOWS = N_PAD // WIN            # 49
CHUNKS_PER_WIN = WIN // 128         # 4
FEAT_CHUNKS = D_FEAT // 128         # 16
NC_HALF = N_CLASSES // 2            # 500 (<=512 moving-free-dim limit)


def build_kernel(
    num_devices: int = N_CORES,
    d_feat: int = D_FEAT,
    n_pad: int = N_PAD,
    n_classes: int = N_CLASSES,
    win: int = WIN,
    keys_bufs: int = 3,
    vals_bufs: int = 8,
):
    """Builds + compiles the per-core Bass program (SPMD: same program on
    every core; each core receives its own keys/vals shard)."""
    feat_chunks = d_feat // 128
    n_windows = n_pad // win
    chunks_per_win = win // 128
    nc_half = n_classes // 2
    n_chunks = n_pad // 128

    nc = bacc.Bacc(
        "TRN2",
        target_bir_lowering=False,
        debug=False,
        num_devices=num_devices,
    )

    x_d = nc.dram_tensor("x", [1, d_feat], F32, kind="ExternalInput").ap()
    # keys/vals arrive host-retiled AND hi/lo bf16-split; each window is one
    # contiguous block with contiguous per-partition runs:
    #   k{h,l}[w, p, c*win + j]  = bf16 split of keys_shard[c*128+p, w*win+j]
    #   v{h,l}[w, p, q*ncls + j] = bf16 split of vals_shard[(w*cpw+q)*128+p, j]
    kh_d = nc.dram_tensor(
        "kh", [n_windows, 128, feat_chunks * win], BF16, kind="ExternalInput"
    ).ap()
    kl_d = nc.dram_tensor(
        "kl", [n_windows, 128, feat_chunks * win], F8, kind="ExternalInput"
    ).ap()
    # vals as a single fp16 plane: the s-side hi/lo split (M=2) keeps the
    # product at ~2^-11 accuracy, so no vals residual plane is needed.
    vh_d = nc.dram_tensor(
        "vh", [n_windows, 128, chunks_per_win * n_classes], F16,
        kind="ExternalInput",
    ).ap()

    out_d = nc.dram_tensor("out", [1, n_classes], F32, kind="ExternalOutput").ap()

    with tile.TileContext(nc) as tc, ExitStack() as ctx:
        const = ctx.enter_context(tc.tile_pool(name="const", bufs=1))
        keys_pool = ctx.enter_context(tc.tile_pool(name="keys", bufs=keys_bufs))
        vals_pool = ctx.enter_context(tc.tile_pool(name="vals", bufs=vals_bufs))
        s_pool = ctx.enter_context(tc.tile_pool(name="s", bufs=4))
        st_pool = ctx.enter_context(tc.tile_pool(name="st", bufs=4))
        psum_s = ctx.enter_context(tc.tile_pool(name="psum_s", bufs=3, space="PSUM"))
        psum_t = ctx.enter_context(tc.tile_pool(name="psum_t", bufs=2, space="PSUM"))
        psum_p = ctx.enter_context(tc.tile_pool(name="psum_p", bufs=1, space="PSUM"))
        dram = ctx.enter_context(tc.tile_pool(name="dram", bufs=1, space="DRAM"))

        # ---- prologue: xt = x reshaped [128, feat_chunks]; scale = theta/||x||
        xt = const.tile([128, feat_chunks], F32)
        nc.sync.dma_start(out=xt[:], in_=x_d.rearrange("a (c p) -> p (a c)", p=128))

        ones = const.tile([128, 1], F32)
        nc.vector.memset(ones[:], 1.0)

        sq = const.tile([128, feat_chunks], F32)
        nc.vector.tensor_mul(sq[:], xt[:], xt[:])
        sums = const.tile([128, 1], F32)
        nc.vector.tensor_reduce(
            sums[:], sq[:], axis=mybir.AxisListType.X, op=mybir.AluOpType.add
        )
        nrm2_ps = psum_t.tile([1, 1], F32, tag="ps_t")
        nc.tensor.matmul(nrm2_ps[:], lhsT=ones[:], rhs=sums[:], start=True, stop=True)
        nrm = const.tile([1, 1], F32)
        nc.scalar.sqrt(nrm[:], nrm2_ps[:])
        inv = const.tile([1, 1], F32)
        nc.vector.reciprocal(inv[:], nrm[:])
        scale = const.tile([1, 1], F32)
        nc.vector.tensor_scalar_mul(scale[:], inv[:], THETA)
        ones_row = const.tile([1, 2], F32)
        nc.vector.memset(ones_row[:], 1.0)
        sc2_ps = psum_t.tile([2, 1], F32, tag="ps_t")
        nc.tensor.matmul(sc2_ps[:], lhsT=ones_row[:], rhs=scale[:], start=True, stop=True)
        scale2 = const.tile([2, 1], F32)
        nc.vector.tensor_copy(scale2[:], sc2_ps[:])

        # x hi/lo bf16 split, interleaved as xs[:, c, 0]=xh, xs[:, c, 1]=xl
        xh_bf = const.tile([128, feat_chunks], BF16)
        nc.vector.tensor_copy(xh_bf[:], xt[:])
        xh32 = const.tile([128, feat_chunks], F32)
        nc.vector.tensor_copy(xh32[:], xh_bf[:])
        xl32 = const.tile([128, feat_chunks], F32)
        nc.vector.tensor_sub(xl32[:], xt[:], xh32[:])
        xs = const.tile([128, feat_chunks, 2], BF16)
        nc.vector.tensor_copy(xs[:, :, 0:1], xh_bf[:].rearrange("p (c o) -> p c o", o=1))
        nc.vector.tensor_copy(xs[:, :, 1:2], xl32[:].rearrange("p (c o) -> p c o", o=1))
        # x-hi scaled down for the fp8 keys-residual pass (scales cancel in MM)
        xsc = const.tile([128, feat_chunks], F32)
        nc.vector.tensor_scalar_mul(xsc[:], xh32[:], 1.0 / F8_SCALE)
        xfp = const.tile([128, feat_chunks], F8)
        nc.vector.tensor_copy(xfp[:], xsc[:])

        # ---- persistent [2, nc_half] accumulators (row0: hi-part, row1: lo-x part)
        pp_a = psum_p.tile([2, nc_half], F32, tag="pp_a")
        pp_b = psum_p.tile([2, nc_half], F32, tag="pp_b")

        def emit_post(ps_s, vh, w):
            # fused: ps_t[128,1] = scale*(row0+row1) transposed, per 128-chunk
            s2 = s_pool.tile([2, win], F32, tag="s2")
            nc.vector.tensor_copy(s2[:], ps_s[:])
            ss = st_pool.tile([128, chunks_per_win, 2], BF16)
            for q in range(chunks_per_win):
                ps_t = psum_t.tile([128, 1], F32, tag="ps_t")
                nc.tensor.matmul(
                    ps_t[:],
                    lhsT=s2[:, q * 128:(q + 1) * 128],
                    rhs=scale2[:],
                    start=True,
                    stop=True,
                )
                se = st_pool.tile([128, 1], F32, tag="se")
                nc.scalar.activation(se[:], ps_t[:], AF.Exp)
                nc.vector.tensor_copy(ss[:, q, 0:1], se[:])
                sh32 = st_pool.tile([128, 1], F32, tag="sh32")
                nc.vector.tensor_copy(sh32[:], ss[:, q, 0:1])
                sl32 = st_pool.tile([128, 1], F32, tag="sl32")
                nc.vector.tensor_sub(sl32[:], se[:], sh32[:])
                nc.vector.tensor_copy(ss[:, q, 1:2], sl32[:])
            # stage 2: pp[0,:] += sh@V ; pp[1,:] += sl@V   (V is fp16)
            for q in range(chunks_per_win):
                gc = w * chunks_per_win + q
                first = gc == 0
                last = gc == n_chunks - 1
                for pp, j0 in ((pp_a, 0), (pp_b, nc_half)):
                    nc.tensor.matmul(
                        pp[:],
                        lhsT=ss[:, q, :],
                        rhs=vh[:, q, j0:j0 + nc_half],
                        start=first,
                        stop=last,
                        skip_group_check=True,
                    )

        # Software-pipelined emission: window w's post-chain (rowsum/exp/
        # transpose/stage-2) is emitted AFTER window w+1's stage-1 matmuls so
        # the PE stream stays dense while DVE/ACT work on the previous window.
        pend = None
        for w in range(n_windows):
            kh = keys_pool.tile([128, feat_chunks, win], BF16, tag="keys")
            nc.sync.dma_start(
                out=kh[:], in_=kh_d[w].rearrange("p (c j) -> p c j", c=feat_chunks)
            )
            kl = keys_pool.tile([128, feat_chunks, win], F8, tag="keys_lo")
            nc.sync.dma_start(
                out=kl[:], in_=kl_d[w].rearrange("p (c j) -> p c j", c=feat_chunks)
            )
            vh = vals_pool.tile([128, chunks_per_win, n_classes], F16, tag="vals")
            nc.sync.dma_start(
                out=vh[:], in_=vh_d[w].rearrange("p (q j) -> p q j", q=chunks_per_win)
            )

            # stage 1: ps_s[0,:] = xh@Kh + xh@Kl ; ps_s[1,:] = xl@Kh
            ps_s = psum_s.tile([2, win], F32)
            for c in range(feat_chunks):
                nc.tensor.matmul(
                    ps_s[:],
                    lhsT=xs[:, c, :],
                    rhs=kh[:, c, :],
                    start=(c == 0),
                    stop=False,
                    skip_group_check=True,
                )
                nc.tensor.matmul(
                    ps_s[0:1, :],
                    lhsT=xfp[:, c:c + 1],
                    rhs=kl[:, c, :],
                    start=False,
                    stop=(c == feat_chunks - 1),
                    skip_group_check=True,
                )

            if pend is not None:
                emit_post(*pend)
            pend = (ps_s, vh, w)
        emit_post(*pend)

        # ---- tail: p = row0 + row1 (copy to SBUF, K=2 ones-matmul row sum)
        p_sb = const.tile([1, n_classes], F32)
        for pp, j0 in ((pp_a, 0), (pp_b, nc_half)):
            pc = const.tile([2, nc_half], F32, tag=f"pc{j0}")
            nc.vector.tensor_copy(pc[:], pp[:])
            pr = psum_t.tile([1, nc_half], F32, tag="ps_t")
            nc.tensor.matmul(
                pr[:], lhsT=ones[0:2, 0:1], rhs=pc[:], start=True, stop=True
            )
            nc.vector.tensor_copy(p_sb[:, j0:j0 + nc_half], pr[:])

        partial = dram.tile([1, n_classes], F32)
        reduced = dram.tile([1, n_classes], F32)
        nc.gpsimd.dma_start(partial[:], p_sb[:])
        nc.gpsimd.collective_compute(
            "AllReduce",
            mybir.AluOpType.add,
            replica_groups=[list(range(num_devices))],
            ins=[partial.opt()],
            outs=[reduced.opt()],
        )
        red_sb = const.tile([1, n_classes], F32)
        nc.sync.dma_start(red_sb[:], reduced[:])
        logp = const.tile([1, n_classes], F32)
        nc.scalar.activation(logp[:], red_sb[:], AF.Ln)
        nc.sync.dma_start(out_d[:], logp[:])

    nc.compile()
    return nc


_NC_CACHE: dict = {}


def _get_nc():
    if "nc" not in _NC_CACHE:
        _NC_CACHE["nc"] = build_kernel()
    return _NC_CACHE["nc"]


def _split_hi_lo(a):
    hi = a.astype(BF16_NP)
    lo = (a - hi.astype(np.float32)).astype(BF16_NP)
    return hi, lo


def _retile_keys(keys_shard, feat_chunks=FEAT_CHUNKS, win=WIN):
    """[d_feat, n_pad] -> [n_windows, 128, feat_chunks*win] with
    out[w, p, c*win + j] = keys_shard[c*128 + p, w*win + j]."""
    d_feat, n_pad = keys_shard.shape
    n_windows = n_pad // win
    v = keys_shard.reshape(feat_chunks, 128, n_windows, win)
    return np.ascontiguousarray(v.transpose(2, 1, 0, 3)).reshape(
        n_windows, 128, feat_chunks * win
    )


def _retile_vals(vals_shard, chunks_per_win=CHUNKS_PER_WIN, win=WIN):
    """[n_pad, n_classes] -> [n_windows, 128, chunks_per_win*n_classes] with
    out[w, p, q*ncls + j] = vals_shard[(w*cpw + q)*128 + p, j]."""
    n_pad, ncls = vals_shard.shape
    n_windows = n_pad // win
    v = vals_shard.reshape(n_windows, chunks_per_win, 128, ncls)
    return np.ascontiguousarray(v.transpose(0, 2, 1, 3)).reshape(
        n_windows, 128, chunks_per_win * ncls
    )


def _shard_inputs(x, mem_keys, mem_vals):
    x = np.ascontiguousarray(np.asarray(x, dtype=np.float32))
    in_maps = []
    for i in range(N_CORES):
        lo_i, hi_i = i * N_SHARD, (i + 1) * N_SHARD
        keys_shard = np.zeros((D_FEAT, N_PAD), dtype=np.float32)
        keys_shard[:, :N_SHARD] = mem_keys[:, lo_i:hi_i]
        vals_shard = np.zeros((N_PAD, N_CLASSES), dtype=np.float32)
        vals_shard[:N_SHARD, :] = mem_vals[lo_i:hi_i, :]
        kt = _retile_keys(keys_shard)
        kh = kt.astype(BF16_NP)
        kl = ((kt - kh.astype(np.float32)) * F8_SCALE).astype(F8_NP)
        vh = _retile_vals(vals_shard).astype(np.float16)
        in_maps.append({"x": x, "kh": kh, "kl": kl, "vh": vh})
    return in_maps


def run(x, mem_keys, mem_vals, trace: bool = False):
    """Runs the SPMD kernel; returns (output [1, N_CLASSES], BassKernelResults)."""
    from concourse.bass_utils import run_bass_kernel_spmd

    nc = _get_nc()
    in_maps = _shard_inputs(x, mem_keys, mem_vals)
    res = run_bass_kernel_spmd(nc, in_maps, list(range(N_CORES)), trace=trace)
    out = np.asarray(res.results[0]["out"], dtype=np.float32).reshape(1, N_CLASSES)
    return out, res


def kernel(x, mem_keys, mem_vals):
    out, _ = run(x, mem_keys, mem_vals, trace=False)
    return out


# revision 28
# speedup vs baseline: 1.1524x; 1.0351x over previous
"""Trainium2 Bass kernel for nn_CacheModel (retrieval_knn).

Computes out = log(exp(theta * (x/||x||) @ mem_keys) @ mem_vals) on 8
NeuronCores.  mem_keys is sharded column-wise and mem_vals row-wise over
the N_mem axis; each core computes its partial [1,1000] product, an
on-device AllReduce sums the partials, and each core takes the log.

Precision strategy: fp32 matmuls on trn2 lower to 2 hardware passes AND
do not register as PE activity for the HAM clock gate (PE stuck at
1.2 GHz).  Instead we ship each fp32 operand as a (hi, lo) split pair
and compute a @ b ~= [ah al]@bh + ah@bl with an M=2 stationary trick —
same PE cycles as fp32, but at the full 2.4 GHz warm clock.  vals use
bf16+bf16 (~2^-16 rel); keys use bf16 hi + *fp8e4m3* lo scaled by 16
(x-hi is pre-divided by 16 so the scales cancel inside the matmul),
cutting keys DMA bytes 4B->3B per element at ~2^-13 rel accuracy.

Self-contained: hardcodes all shapes; imports only the system-installed
concourse stack + numpy.
"""

from contextlib import ExitStack

import ml_dtypes
import numpy as np

import concourse.bass as bass
import concourse.tile as tile
from concourse import bacc, mybir

F32 = mybir.dt.float32
BF16 = mybir.dt.bfloat16
F8 = mybir.dt.float8e4
F8L = mybir.dt.float8e5
F16 = mybir.dt.float16
AF = mybir.ActivationFunctionType
BF16_NP = ml_dtypes.bfloat16
F8_NP = ml_dtypes.float8_e4m3
F8L_NP = ml_dtypes.float8_e5m2
F8_SCALE = 16.0  # keys-lo residual premultiplied by this; x-hi divided by it

# Problem shapes (full)
D_FEAT = 2048
N_MEM = 200000
N_CLASSES = 1000
THETA = 5.0
N_CORES = 8

# Per-core sharding: 25000 n-rows, zero-padded to 25088 = 196*128 = 49*512
N_SHARD = N_MEM // N_CORES          # 25000
WIN = 512                           # n-window width (one psum bank of f32)
N_PAD = 25088                       # 49 windows * 512
N_WINDOWS = N_PAD // WIN            # 49
CHUNKS_PER_WIN = WIN // 128         # 4
FEAT_CHUNKS = D_FEAT // 128         # 16
NC_HALF = N_CLASSES // 2            # 500 (<=512 moving-free-dim limit)


def build_kernel(
    num_devices: int = N_CORES,
    d_feat: int = D_FEAT,
    n_pad: int = N_PAD,
    n_classes: int = N_CLASSES,
    win: int = WIN,
    keys_bufs: int = 4,
    vals_bufs: int = 8,
):
    """Builds + compiles the per-core Bass program (SPMD: same program on
    every core; each core receives its own keys/vals shard)."""
    feat_chunks = d_feat // 128
    n_windows = n_pad // win
    chunks_per_win = win // 128
    nc_half = n_classes // 2
    n_chunks = n_pad // 128

    nc = bacc.Bacc(
        "TRN2",
        target_bir_lowering=False,
        debug=False,
        num_devices=num_devices,
    )

    x_d = nc.dram_tensor("x", [1, d_feat], F32, kind="ExternalInput").ap()
    # keys/vals arrive host-retiled AND hi/lo bf16-split; each window is one
    # contiguous block with contiguous per-partition runs:
    #   k{h,l}[w, p, c*win + j]  = bf16 split of keys_shard[c*128+p, w*win+j]
    #   v{h,l}[w, p, q*ncls + j] = bf16 split of vals_shard[(w*cpw+q)*128+p, j]
    # keys as a single fp16 plane: x stays a bf16 hi/lo pair (M=2), so the
    # dot-product error is set by the keys' fp16 rounding (~2^-12 rms).
    kh_d = nc.dram_tensor(
        "kh", [n_windows, 128, feat_chunks * win], F16, kind="ExternalInput"
    ).ap()
    # vals as a single fp16 plane: the s-side hi/lo split (M=2) keeps the
    # product at ~2^-11 accuracy, so no vals residual plane is needed.
    vh_d = nc.dram_tensor(
        "vh", [n_windows, 128, chunks_per_win * n_classes], F16,
        kind="ExternalInput",
    ).ap()

    out_d = nc.dram_tensor("out", [1, n_classes], F32, kind="ExternalOutput").ap()

    with tile.TileContext(nc) as tc, ExitStack() as ctx:
        const = ctx.enter_context(tc.tile_pool(name="const", bufs=1))
        keys_pool = ctx.enter_context(tc.tile_pool(name="keys", bufs=keys_bufs))
        vals_pool = ctx.enter_context(tc.tile_pool(name="vals", bufs=vals_bufs))
        s_pool = ctx.enter_context(tc.tile_pool(name="s", bufs=4))
        st_pool = ctx.enter_context(tc.tile_pool(name="st", bufs=4))
        psum_s = ctx.enter_context(tc.tile_pool(name="psum_s", bufs=3, space="PSUM"))
        psum_t = ctx.enter_context(tc.tile_pool(name="psum_t", bufs=2, space="PSUM"))
        psum_p = ctx.enter_context(tc.tile_pool(name="psum_p", bufs=1, space="PSUM"))
        dram = ctx.enter_context(tc.tile_pool(name="dram", bufs=1, space="DRAM"))

        # ---- prologue: xt = x reshaped [128, feat_chunks]; scale = theta/||x||
        xt = const.tile([128, feat_chunks], F32)
        nc.sync.dma_start(out=xt[:], in_=x_d.rearrange("a (c p) -> p (a c)", p=128))

        ones = const.tile([128, 1], F32)
        nc.vector.memset(ones[:], 1.0)

        sq = const.tile([128, feat_chunks], F32)
        nc.vector.tensor_mul(sq[:], xt[:], xt[:])
        sums = const.tile([128, 1], F32)
        nc.vector.tensor_reduce(
            sums[:], sq[:], axis=mybir.AxisListType.X, op=mybir.AluOpType.add
        )
        nrm2_ps = psum_t.tile([1, 1], F32, tag="ps_t")
        nc.tensor.matmul(nrm2_ps[:], lhsT=ones[:], rhs=sums[:], start=True, stop=True)
        nrm = const.tile([1, 1], F32)
        nc.scalar.sqrt(nrm[:], nrm2_ps[:])
        inv = const.tile([1, 1], F32)
        nc.vector.reciprocal(inv[:], nrm[:])
        scale = const.tile([1, 1], F32)
        nc.vector.tensor_scalar_mul(scale[:], inv[:], THETA)
        ones_row = const.tile([1, 2], F32)
        nc.vector.memset(ones_row[:], 1.0)
        sc2_ps = psum_t.tile([2, 1], F32, tag="ps_t")
        nc.tensor.matmul(sc2_ps[:], lhsT=ones_row[:], rhs=scale[:], start=True, stop=True)
        scale2 = const.tile([2, 1], F32)
        nc.vector.tensor_copy(scale2[:], sc2_ps[:])

        # x hi/lo bf16 split, interleaved as xs[:, c, 0]=xh, xs[:, c, 1]=xl
        xh_bf = const.tile([128, feat_chunks], BF16)
        nc.vector.tensor_copy(xh_bf[:], xt[:])
        xh32 = const.tile([128, feat_chunks], F32)
        nc.vector.tensor_copy(xh32[:], xh_bf[:])
        xl32 = const.tile([128, feat_chunks], F32)
        nc.vector.tensor_sub(xl32[:], xt[:], xh32[:])
        xs = const.tile([128, feat_chunks, 2], BF16)
        nc.vector.tensor_copy(xs[:, :, 0:1], xh_bf[:].rearrange("p (c o) -> p c o", o=1))
        nc.vector.tensor_copy(xs[:, :, 1:2], xl32[:].rearrange("p (c o) -> p c o", o=1))

        # ---- persistent [2, nc_half] accumulators (row0: hi-part, row1: lo-x part)
        pp_a = psum_p.tile([2, nc_half], F32, tag="pp_a")
        pp_b = psum_p.tile([2, nc_half], F32, tag="pp_b")

        def emit_post(ps_s, vh, w):
            # fused: ps_t[128,1] = scale*(row0+row1) transposed, per 128-chunk
            s2 = s_pool.tile([2, win], F32, tag="s2")
            nc.vector.tensor_copy(s2[:], ps_s[:])
            ss = st_pool.tile([128, chunks_per_win, 2], BF16)
            for q in range(chunks_per_win):
                ps_t = psum_t.tile([128, 1], F32, tag="ps_t")
                nc.tensor.matmul(
                    ps_t[:],
                    lhsT=s2[:, q * 128:(q + 1) * 128],
                    rhs=scale2[:],
                    start=True,
                    stop=True,
                )
                se = st_pool.tile([128, 1], F32, tag="se")
                nc.scalar.activation(se[:], ps_t[:], AF.Exp)
                nc.vector.tensor_copy(ss[:, q, 0:1], se[:])
                sh32 = st_pool.tile([128, 1], F32, tag="sh32")
                nc.vector.tensor_copy(sh32[:], ss[:, q, 0:1])
                sl32 = st_pool.tile([128, 1], F32, tag="sl32")
                nc.vector.tensor_sub(sl32[:], se[:], sh32[:])
                nc.vector.tensor_copy(ss[:, q, 1:2], sl32[:])
            # stage 2: pp[0,:] += sh@V ; pp[1,:] += sl@V   (V is fp16)
            for q in range(chunks_per_win):
                gc = w * chunks_per_win + q
                first = gc == 0
                last = gc == n_chunks - 1
                for pp, j0 in ((pp_a, 0), (pp_b, nc_half)):
                    nc.tensor.matmul(
                        pp[:],
                        lhsT=ss[:, q, :],
                        rhs=vh[:, q, j0:j0 + nc_half],
                        start=first,
                        stop=last,
                        skip_group_check=True,
                    )

        # Software-pipelined emission: window w's post-chain (rowsum/exp/
        # transpose/stage-2) is emitted AFTER window w+1's stage-1 matmuls so
        # the PE stream stays dense while DVE/ACT work on the previous window.
        pend = None
        for w in range(n_windows):
            kh = keys_pool.tile([128, feat_chunks, win], F16, tag="keys")
            nc.sync.dma_start(
                out=kh[:], in_=kh_d[w].rearrange("p (c j) -> p c j", c=feat_chunks)
            )
            vh = vals_pool.tile([128, chunks_per_win, n_classes], F16, tag="vals")
            nc.sync.dma_start(
                out=vh[:], in_=vh_d[w].rearrange("p (q j) -> p q j", q=chunks_per_win)
            )

            # stage 1: ps_s[0,:] = xh@K ; ps_s[1,:] = xl@K   (K is fp16)
            ps_s = psum_s.tile([2, win], F32)
            for c in range(feat_chunks):
                nc.tensor.matmul(
                    ps_s[:],
                    lhsT=xs[:, c, :],
                    rhs=kh[:, c, :],
                    start=(c == 0),
                    stop=(c == feat_chunks - 1),
                    skip_group_check=True,
                )

            if pend is not None:
                emit_post(*pend)
            pend = (ps_s, vh, w)
        emit_post(*pend)

        # ---- tail: p = row0 + row1 (copy to SBUF, K=2 ones-matmul row sum)
        p_sb = const.tile([1, n_classes], F32)
        for pp, j0 in ((pp_a, 0), (pp_b, nc_half)):
            pc = const.tile([2, nc_half], F32, tag=f"pc{j0}")
            nc.vector.tensor_copy(pc[:], pp[:])
            pr = psum_t.tile([1, nc_half], F32, tag="ps_t")
            nc.tensor.matmul(
                pr[:], lhsT=ones[0:2, 0:1], rhs=pc[:], start=True, stop=True
            )
            nc.vector.tensor_copy(p_sb[:, j0:j0 + nc_half], pr[:])

        partial = dram.tile([1, n_classes], F32)
        reduced = dram.tile([1, n_classes], F32)
        nc.gpsimd.dma_start(partial[:], p_sb[:])
        nc.gpsimd.collective_compute(
            "AllReduce",
            mybir.AluOpType.add,
            replica_groups=[list(range(num_devices))],
            ins=[partial.opt()],
            outs=[reduced.opt()],
        )
        red_sb = const.tile([1, n_classes], F32)
        nc.sync.dma_start(red_sb[:], reduced[:])
        logp = const.tile([1, n_classes], F32)
        nc.scalar.activation(logp[:], red_sb[:], AF.Ln)
        nc.sync.dma_start(out_d[:], logp[:])

    nc.compile()
    return nc


_NC_CACHE: dict = {}


def _get_nc():
    if "nc" not in _NC_CACHE:
        _NC_CACHE["nc"] = build_kernel()
    return _NC_CACHE["nc"]


def _split_hi_lo(a):
    hi = a.astype(BF16_NP)
    lo = (a - hi.astype(np.float32)).astype(BF16_NP)
    return hi, lo


def _retile_keys(keys_shard, feat_chunks=FEAT_CHUNKS, win=WIN):
    """[d_feat, n_pad] -> [n_windows, 128, feat_chunks*win] with
    out[w, p, c*win + j] = keys_shard[c*128 + p, w*win + j]."""
    d_feat, n_pad = keys_shard.shape
    n_windows = n_pad // win
    v = keys_shard.reshape(feat_chunks, 128, n_windows, win)
    return np.ascontiguousarray(v.transpose(2, 1, 0, 3)).reshape(
        n_windows, 128, feat_chunks * win
    )


def _retile_vals(vals_shard, chunks_per_win=CHUNKS_PER_WIN, win=WIN):
    """[n_pad, n_classes] -> [n_windows, 128, chunks_per_win*n_classes] with
    out[w, p, q*ncls + j] = vals_shard[(w*cpw + q)*128 + p, j]."""
    n_pad, ncls = vals_shard.shape
    n_windows = n_pad // win
    v = vals_shard.reshape(n_windows, chunks_per_win, 128, ncls)
    return np.ascontiguousarray(v.transpose(0, 2, 1, 3)).reshape(
        n_windows, 128, chunks_per_win * ncls
    )


def _shard_inputs(x, mem_keys, mem_vals):
    x = np.ascontiguousarray(np.asarray(x, dtype=np.float32))
    in_maps = []
    for i in range(N_CORES):
        lo_i, hi_i = i * N_SHARD, (i + 1) * N_SHARD
        keys_shard = np.zeros((D_FEAT, N_PAD), dtype=np.float32)
        keys_shard[:, :N_SHARD] = mem_keys[:, lo_i:hi_i]
        vals_shard = np.zeros((N_PAD, N_CLASSES), dtype=np.float32)
        vals_shard[:N_SHARD, :] = mem_vals[lo_i:hi_i, :]
        kh = _retile_keys(keys_shard).astype(np.float16)
        vh = _retile_vals(vals_shard).astype(np.float16)
        in_maps.append({"x": x, "kh": kh, "vh": vh})
    return in_maps


def run(x, mem_keys, mem_vals, trace: bool = False):
    """Runs the SPMD kernel; returns (output [1, N_CLASSES], BassKernelResults)."""
    from concourse.bass_utils import run_bass_kernel_spmd

    nc = _get_nc()
    in_maps = _shard_inputs(x, mem_keys, mem_vals)
    res = run_bass_kernel_spmd(nc, in_maps, list(range(N_CORES)), trace=trace)
    out = np.asarray(res.results[0]["out"], dtype=np.float32).reshape(1, N_CLASSES)
    return out, res


def kernel(x, mem_keys, mem_vals):
    out, _ = run(x, mem_keys, mem_vals, trace=False)
    return out


# revision 29
# speedup vs baseline: 1.4606x; 1.2674x over previous
"""Trainium2 Bass kernel for nn_CacheModel (retrieval_knn).

Computes out = log(exp(theta * (x/||x||) @ mem_keys) @ mem_vals) on 8
NeuronCores.  mem_keys is sharded column-wise and mem_vals row-wise over
the N_mem axis; each core computes its partial [1,1000] product, an
on-device AllReduce sums the partials, and each core takes the log.

Precision strategy: fp32 matmuls on trn2 lower to 2 hardware passes AND
do not register as PE activity for the HAM clock gate (PE stuck at
1.2 GHz).  Instead we ship each fp32 operand as a (hi, lo) split pair
and compute a @ b ~= [ah al]@bh + ah@bl with an M=2 stationary trick —
same PE cycles as fp32, but at the full 2.4 GHz warm clock.  vals use
bf16+bf16 (~2^-16 rel); keys use bf16 hi + *fp8e4m3* lo scaled by 16
(x-hi is pre-divided by 16 so the scales cancel inside the matmul),
cutting keys DMA bytes 4B->3B per element at ~2^-13 rel accuracy.

Self-contained: hardcodes all shapes; imports only the system-installed
concourse stack + numpy.
"""

from contextlib import ExitStack

import ml_dtypes
import numpy as np

import concourse.bass as bass
import concourse.tile as tile
from concourse import bacc, mybir

F32 = mybir.dt.float32
BF16 = mybir.dt.bfloat16
F8 = mybir.dt.float8e4
F8L = mybir.dt.float8e5
F16 = mybir.dt.float16
AF = mybir.ActivationFunctionType
BF16_NP = ml_dtypes.bfloat16
F8_NP = ml_dtypes.float8_e4m3
F8L_NP = ml_dtypes.float8_e5m2
F8_SCALE = 16.0  # keys-lo residual premultiplied by this; x-hi divided by it

# Problem shapes (full)
D_FEAT = 2048
N_MEM = 200000
N_CLASSES = 1000
THETA = 5.0
N_CORES = 8

# Per-core sharding: 25000 n-rows, zero-padded to 25088 = 196*128 = 49*512
N_SHARD = N_MEM // N_CORES          # 25000
WIN = 512                           # n-window width (one psum bank of f32)
N_PAD = 25088                       # 49 windows * 512
N_WINDOWS = N_PAD // WIN            # 49
CHUNKS_PER_WIN = WIN // 128         # 4
FEAT_CHUNKS = D_FEAT // 128         # 16
NC_HALF = N_CLASSES // 2            # 500 (<=512 moving-free-dim limit)


def build_kernel(
    num_devices: int = N_CORES,
    d_feat: int = D_FEAT,
    n_pad: int = N_PAD,
    n_classes: int = N_CLASSES,
    win: int = WIN,
    keys_bufs: int = 4,
    vals_bufs: int = 8,
):
    """Builds + compiles the per-core Bass program (SPMD: same program on
    every core; each core receives its own keys/vals shard)."""
    feat_chunks = d_feat // 128
    n_windows = n_pad // win
    chunks_per_win = win // 128
    nc_half = n_classes // 2
    n_chunks = n_pad // 128

    nc = bacc.Bacc(
        "TRN2",
        target_bir_lowering=False,
        debug=False,
        num_devices=num_devices,
    )

    x_d = nc.dram_tensor("x", [1, d_feat], F32, kind="ExternalInput").ap()
    # keys/vals arrive host-retiled AND hi/lo bf16-split; each window is one
    # contiguous block with contiguous per-partition runs:
    #   k{h,l}[w, p, c*win + j]  = bf16 split of keys_shard[c*128+p, w*win+j]
    #   v{h,l}[w, p, q*ncls + j] = bf16 split of vals_shard[(w*cpw+q)*128+p, j]
    # keys as a single fp16 plane: x stays a bf16 hi/lo pair (M=2), so the
    # dot-product error is set by the keys' fp16 rounding (~2^-12 rms).
    kh_d = nc.dram_tensor(
        "kh", [n_windows, 128, feat_chunks * win], F16, kind="ExternalInput"
    ).ap()
    # vals as a single fp16 plane: the s-side hi/lo split (M=2) keeps the
    # product at ~2^-11 accuracy, so no vals residual plane is needed.
    vh_d = nc.dram_tensor(
        "vh", [n_windows, 128, chunks_per_win * n_classes], F16,
        kind="ExternalInput",
    ).ap()

    out_d = nc.dram_tensor("out", [1, n_classes], F32, kind="ExternalOutput").ap()

    with tile.TileContext(nc) as tc, ExitStack() as ctx:
        const = ctx.enter_context(tc.tile_pool(name="const", bufs=1))
        keys_pool = ctx.enter_context(tc.tile_pool(name="keys", bufs=keys_bufs))
        vals_pool = ctx.enter_context(tc.tile_pool(name="vals", bufs=vals_bufs))
        s_pool = ctx.enter_context(tc.tile_pool(name="s", bufs=4))
        st_pool = ctx.enter_context(tc.tile_pool(name="st", bufs=4))
        psum_s = ctx.enter_context(tc.tile_pool(name="psum_s", bufs=3, space="PSUM"))
        psum_t = ctx.enter_context(tc.tile_pool(name="psum_t", bufs=2, space="PSUM"))
        psum_p = ctx.enter_context(tc.tile_pool(name="psum_p", bufs=1, space="PSUM"))
        dram = ctx.enter_context(tc.tile_pool(name="dram", bufs=1, space="DRAM"))

        # ---- prologue: xt = x reshaped [128, feat_chunks]; scale = theta/||x||
        xt = const.tile([128, feat_chunks], F32)
        nc.sync.dma_start(out=xt[:], in_=x_d.rearrange("a (c p) -> p (a c)", p=128))

        ones = const.tile([128, 1], F32)
        nc.vector.memset(ones[:], 1.0)

        sq = const.tile([128, feat_chunks], F32)
        nc.vector.tensor_mul(sq[:], xt[:], xt[:])
        sums = const.tile([128, 1], F32)
        nc.vector.tensor_reduce(
            sums[:], sq[:], axis=mybir.AxisListType.X, op=mybir.AluOpType.add
        )
        nrm2_ps = psum_t.tile([1, 1], F32, tag="ps_t")
        nc.tensor.matmul(nrm2_ps[:], lhsT=ones[:], rhs=sums[:], start=True, stop=True)
        nrm = const.tile([1, 1], F32)
        nc.scalar.sqrt(nrm[:], nrm2_ps[:])
        inv = const.tile([1, 1], F32)
        nc.vector.reciprocal(inv[:], nrm[:])
        scale = const.tile([1, 1], F32)
        nc.vector.tensor_scalar_mul(scale[:], inv[:], THETA)
        ones_row = const.tile([1, 2], F32)
        nc.vector.memset(ones_row[:], 1.0)
        sc2_ps = psum_t.tile([2, 1], F32, tag="ps_t")
        nc.tensor.matmul(sc2_ps[:], lhsT=ones_row[:], rhs=scale[:], start=True, stop=True)
        scale2 = const.tile([2, 1], F32)
        nc.vector.tensor_copy(scale2[:], sc2_ps[:])

        # x hi/lo bf16 split, interleaved as xs[:, c, 0]=xh, xs[:, c, 1]=xl
        xh_bf = const.tile([128, feat_chunks], BF16)
        nc.vector.tensor_copy(xh_bf[:], xt[:])
        xh32 = const.tile([128, feat_chunks], F32)
        nc.vector.tensor_copy(xh32[:], xh_bf[:])
        xl32 = const.tile([128, feat_chunks], F32)
        nc.vector.tensor_sub(xl32[:], xt[:], xh32[:])
        xs = const.tile([128, feat_chunks, 2], BF16)
        nc.vector.tensor_copy(xs[:, :, 0:1], xh_bf[:].rearrange("p (c o) -> p c o", o=1))
        nc.vector.tensor_copy(xs[:, :, 1:2], xl32[:].rearrange("p (c o) -> p c o", o=1))

        # ---- persistent [2, nc_half] accumulators (row0: hi-part, row1: lo-x part)
        pp_a = psum_p.tile([2, nc_half], F32, tag="pp_a")
        pp_b = psum_p.tile([2, nc_half], F32, tag="pp_b")

        def emit_post(ps_s, vh, w):
            # fused: ps_t[128,1] = scale*(row0+row1) transposed, per 128-chunk
            s2 = s_pool.tile([2, win], F32, tag="s2")
            nc.vector.tensor_copy(s2[:], ps_s[:])
            ss = st_pool.tile([128, chunks_per_win, 2], BF16)
            for q in range(chunks_per_win):
                ps_t = psum_t.tile([128, 1], F32, tag="ps_t")
                nc.tensor.matmul(
                    ps_t[:],
                    lhsT=s2[:, q * 128:(q + 1) * 128],
                    rhs=scale2[:],
                    start=True,
                    stop=True,
                )
                se = st_pool.tile([128, 1], F32, tag="se")
                nc.scalar.activation(se[:], ps_t[:], AF.Exp)
                nc.vector.tensor_copy(ss[:, q, 0:1], se[:])
                sh32 = st_pool.tile([128, 1], F32, tag="sh32")
                nc.vector.tensor_copy(sh32[:], ss[:, q, 0:1])
                sl32 = st_pool.tile([128, 1], F32, tag="sl32")
                nc.vector.tensor_sub(sl32[:], se[:], sh32[:])
                nc.vector.tensor_copy(ss[:, q, 1:2], sl32[:])
            # stage 2: pp[0,:] += sh@V ; pp[1,:] += sl@V   (V is fp16)
            for q in range(chunks_per_win):
                gc = w * chunks_per_win + q
                first = gc == 0
                last = gc == n_chunks - 1
                for pp, j0 in ((pp_a, 0), (pp_b, nc_half)):
                    nc.tensor.matmul(
                        pp[:],
                        lhsT=ss[:, q, :],
                        rhs=vh[:, q, j0:j0 + nc_half],
                        start=first,
                        stop=last,
                        skip_group_check=True,
                    )

        # Software-pipelined emission, depth 2: window w's post-chain
        # (rowsum/exp/transpose/stage-2) is emitted after window w+2's
        # stage-1 matmuls, giving the ACT/DVE exp+cast chain a full extra
        # window to complete before the PE needs its stage-2 operands.
        pends = []
        for w in range(n_windows):
            kh = keys_pool.tile([128, feat_chunks, win], F16, tag="keys")
            nc.sync.dma_start(
                out=kh[:], in_=kh_d[w].rearrange("p (c j) -> p c j", c=feat_chunks)
            )
            vh = vals_pool.tile([128, chunks_per_win, n_classes], F16, tag="vals")
            nc.sync.dma_start(
                out=vh[:], in_=vh_d[w].rearrange("p (q j) -> p q j", q=chunks_per_win)
            )

            # stage 1: ps_s[0,:] = xh@K ; ps_s[1,:] = xl@K   (K is fp16)
            ps_s = psum_s.tile([2, win], F32)
            for c in range(feat_chunks):
                nc.tensor.matmul(
                    ps_s[:],
                    lhsT=xs[:, c, :],
                    rhs=kh[:, c, :],
                    start=(c == 0),
                    stop=(c == feat_chunks - 1),
                    skip_group_check=True,
                )

            pends.append((ps_s, vh, w))
            if len(pends) > 2:
                emit_post(*pends.pop(0))
        for p in pends:
            emit_post(*p)

        # ---- tail: p = row0 + row1 (copy to SBUF, K=2 ones-matmul row sum)
        p_sb = const.tile([1, n_classes], F32)
        for pp, j0 in ((pp_a, 0), (pp_b, nc_half)):
            pc = const.tile([2, nc_half], F32, tag=f"pc{j0}")
            nc.vector.tensor_copy(pc[:], pp[:])
            pr = psum_t.tile([1, nc_half], F32, tag="ps_t")
            nc.tensor.matmul(
                pr[:], lhsT=ones[0:2, 0:1], rhs=pc[:], start=True, stop=True
            )
            nc.vector.tensor_copy(p_sb[:, j0:j0 + nc_half], pr[:])

        partial = dram.tile([1, n_classes], F32)
        reduced = dram.tile([1, n_classes], F32)
        nc.gpsimd.dma_start(partial[:], p_sb[:])
        nc.gpsimd.collective_compute(
            "AllReduce",
            mybir.AluOpType.add,
            replica_groups=[list(range(num_devices))],
            ins=[partial.opt()],
            outs=[reduced.opt()],
        )
        red_sb = const.tile([1, n_classes], F32)
        nc.sync.dma_start(red_sb[:], reduced[:])
        logp = const.tile([1, n_classes], F32)
        nc.scalar.activation(logp[:], red_sb[:], AF.Ln)
        nc.sync.dma_start(out_d[:], logp[:])

    nc.compile()
    return nc


_NC_CACHE: dict = {}


def _get_nc():
    if "nc" not in _NC_CACHE:
        _NC_CACHE["nc"] = build_kernel()
    return _NC_CACHE["nc"]


def _split_hi_lo(a):
    hi = a.astype(BF16_NP)
    lo = (a - hi.astype(np.float32)).astype(BF16_NP)
    return hi, lo


def _retile_keys(keys_shard, feat_chunks=FEAT_CHUNKS, win=WIN):
    """[d_feat, n_pad] -> [n_windows, 128, feat_chunks*win] with
    out[w, p, c*win + j] = keys_shard[c*128 + p, w*win + j]."""
    d_feat, n_pad = keys_shard.shape
    n_windows = n_pad // win
    v = keys_shard.reshape(feat_chunks, 128, n_windows, win)
    return np.ascontiguousarray(v.transpose(2, 1, 0, 3)).reshape(
        n_windows, 128, feat_chunks * win
    )


def _retile_vals(vals_shard, chunks_per_win=CHUNKS_PER_WIN, win=WIN):
    """[n_pad, n_classes] -> [n_windows, 128, chunks_per_win*n_classes] with
    out[w, p, q*ncls + j] = vals_shard[(w*cpw + q)*128 + p, j]."""
    n_pad, ncls = vals_shard.shape
    n_windows = n_pad // win
    v = vals_shard.reshape(n_windows, chunks_per_win, 128, ncls)
    return np.ascontiguousarray(v.transpose(0, 2, 1, 3)).reshape(
        n_windows, 128, chunks_per_win * ncls
    )


def _shard_inputs(x, mem_keys, mem_vals):
    x = np.ascontiguousarray(np.asarray(x, dtype=np.float32))
    in_maps = []
    for i in range(N_CORES):
        lo_i, hi_i = i * N_SHARD, (i + 1) * N_SHARD
        keys_shard = np.zeros((D_FEAT, N_PAD), dtype=np.float32)
        keys_shard[:, :N_SHARD] = mem_keys[:, lo_i:hi_i]
        vals_shard = np.zeros((N_PAD, N_CLASSES), dtype=np.float32)
        vals_shard[:N_SHARD, :] = mem_vals[lo_i:hi_i, :]
        kh = _retile_keys(keys_shard).astype(np.float16)
        vh = _retile_vals(vals_shard).astype(np.float16)
        in_maps.append({"x": x, "kh": kh, "vh": vh})
    return in_maps


def run(x, mem_keys, mem_vals, trace: bool = False):
    """Runs the SPMD kernel; returns (output [1, N_CLASSES], BassKernelResults)."""
    from concourse.bass_utils import run_bass_kernel_spmd

    nc = _get_nc()
    in_maps = _shard_inputs(x, mem_keys, mem_vals)
    res = run_bass_kernel_spmd(nc, in_maps, list(range(N_CORES)), trace=trace)
    out = np.asarray(res.results[0]["out"], dtype=np.float32).reshape(1, N_CLASSES)
    return out, res


def kernel(x, mem_keys, mem_vals):
    out, _ = run(x, mem_keys, mem_vals, trace=False)
    return out


# revision 30
# speedup vs baseline: 1.4649x; 1.0029x over previous
"""Trainium2 Bass kernel for nn_CacheModel (retrieval_knn).

Computes out = log(exp(theta * (x/||x||) @ mem_keys) @ mem_vals) on 8
NeuronCores.  mem_keys is sharded column-wise and mem_vals row-wise over
the N_mem axis; each core computes its partial [1,1000] product, an
on-device AllReduce sums the partials, and each core takes the log.

Precision strategy: fp32 matmuls on trn2 lower to 2 hardware passes AND
do not register as PE activity for the HAM clock gate (PE stuck at
1.2 GHz).  Instead: keys and vals ship as single fp16 planes (halving
DMA bytes vs fp32), while the query x is a bf16 (hi, lo) pair used as
an M=2 stationary so its split costs no extra matmuls.  fp16's 10-bit
mantissa keeps the measured output absmax at ~4e-4.  The emission is
software-pipelined at depth 2 (window w's rowsum/exp/transpose/stage-2
chain is emitted after window w+2's stage-1) so the PE never stalls on
the cross-engine exp/cast handoffs.

Self-contained: hardcodes all shapes; imports only the system-installed
concourse stack + numpy.
"""

from contextlib import ExitStack

import ml_dtypes
import numpy as np

import concourse.bass as bass
import concourse.tile as tile
from concourse import bacc, mybir

F32 = mybir.dt.float32
BF16 = mybir.dt.bfloat16
F8 = mybir.dt.float8e4
F8L = mybir.dt.float8e5
F16 = mybir.dt.float16
AF = mybir.ActivationFunctionType
BF16_NP = ml_dtypes.bfloat16
F8_NP = ml_dtypes.float8_e4m3
F8L_NP = ml_dtypes.float8_e5m2
F8_SCALE = 16.0  # keys-lo residual premultiplied by this; x-hi divided by it

# Problem shapes (full)
D_FEAT = 2048
N_MEM = 200000
N_CLASSES = 1000
THETA = 5.0
N_CORES = 8

# Per-core sharding: 25000 n-rows, zero-padded to 25088 = 196*128 = 49*512
N_SHARD = N_MEM // N_CORES          # 25000
WIN = 512                           # n-window width (one psum bank of f32)
N_PAD = 25088                       # 49 windows * 512
N_WINDOWS = N_PAD // WIN            # 49
CHUNKS_PER_WIN = WIN // 128         # 4
FEAT_CHUNKS = D_FEAT // 128         # 16
NC_HALF = N_CLASSES // 2            # 500 (<=512 moving-free-dim limit)


def build_kernel(
    num_devices: int = N_CORES,
    d_feat: int = D_FEAT,
    n_pad: int = N_PAD,
    n_classes: int = N_CLASSES,
    win: int = WIN,
    keys_bufs: int = 4,
    vals_bufs: int = 8,
):
    """Builds + compiles the per-core Bass program (SPMD: same program on
    every core; each core receives its own keys/vals shard)."""
    feat_chunks = d_feat // 128
    n_windows = n_pad // win
    chunks_per_win = win // 128
    nc_half = n_classes // 2
    n_chunks = n_pad // 128

    nc = bacc.Bacc(
        "TRN2",
        target_bir_lowering=False,
        debug=False,
        num_devices=num_devices,
    )

    x_d = nc.dram_tensor("x", [1, d_feat], F32, kind="ExternalInput").ap()
    # keys/vals arrive host-retiled AND hi/lo bf16-split; each window is one
    # contiguous block with contiguous per-partition runs:
    #   k{h,l}[w, p, c*win + j]  = bf16 split of keys_shard[c*128+p, w*win+j]
    #   v{h,l}[w, p, q*ncls + j] = bf16 split of vals_shard[(w*cpw+q)*128+p, j]
    # keys as a single fp16 plane: x stays a bf16 hi/lo pair (M=2), so the
    # dot-product error is set by the keys' fp16 rounding (~2^-12 rms).
    kh_d = nc.dram_tensor(
        "kh", [n_windows, 128, feat_chunks * win], F16, kind="ExternalInput"
    ).ap()
    # vals as a single fp16 plane: the s-side hi/lo split (M=2) keeps the
    # product at ~2^-11 accuracy, so no vals residual plane is needed.
    vh_d = nc.dram_tensor(
        "vh", [n_windows, 128, chunks_per_win * n_classes], F16,
        kind="ExternalInput",
    ).ap()

    out_d = nc.dram_tensor("out", [1, n_classes], F32, kind="ExternalOutput").ap()

    with tile.TileContext(nc) as tc, ExitStack() as ctx:
        const = ctx.enter_context(tc.tile_pool(name="const", bufs=1))
        keys_pool = ctx.enter_context(tc.tile_pool(name="keys", bufs=keys_bufs))
        vals_pool = ctx.enter_context(tc.tile_pool(name="vals", bufs=vals_bufs))
        s_pool = ctx.enter_context(tc.tile_pool(name="s", bufs=4))
        st_pool = ctx.enter_context(tc.tile_pool(name="st", bufs=4))
        psum_s = ctx.enter_context(tc.tile_pool(name="psum_s", bufs=3, space="PSUM"))
        psum_t = ctx.enter_context(tc.tile_pool(name="psum_t", bufs=2, space="PSUM"))
        psum_p = ctx.enter_context(tc.tile_pool(name="psum_p", bufs=1, space="PSUM"))
        dram = ctx.enter_context(tc.tile_pool(name="dram", bufs=1, space="DRAM"))

        # ---- prologue: xt = x reshaped [128, feat_chunks]; scale = theta/||x||
        xt = const.tile([128, feat_chunks], F32)
        nc.sync.dma_start(out=xt[:], in_=x_d.rearrange("a (c p) -> p (a c)", p=128))

        ones = const.tile([128, 1], F32)
        nc.vector.memset(ones[:], 1.0)

        sq = const.tile([128, feat_chunks], F32)
        nc.vector.tensor_mul(sq[:], xt[:], xt[:])
        sums = const.tile([128, 1], F32)
        nc.vector.tensor_reduce(
            sums[:], sq[:], axis=mybir.AxisListType.X, op=mybir.AluOpType.add
        )
        nrm2_ps = psum_t.tile([1, 1], F32, tag="ps_t")
        nc.tensor.matmul(nrm2_ps[:], lhsT=ones[:], rhs=sums[:], start=True, stop=True)
        nrm = const.tile([1, 1], F32)
        nc.scalar.sqrt(nrm[:], nrm2_ps[:])
        inv = const.tile([1, 1], F32)
        nc.vector.reciprocal(inv[:], nrm[:])
        scale = const.tile([1, 1], F32)
        nc.vector.tensor_scalar_mul(scale[:], inv[:], THETA)
        ones_row = const.tile([1, 2], F32)
        nc.vector.memset(ones_row[:], 1.0)
        sc2_ps = psum_t.tile([2, 1], F32, tag="ps_t")
        nc.tensor.matmul(sc2_ps[:], lhsT=ones_row[:], rhs=scale[:], start=True, stop=True)
        scale2 = const.tile([2, 1], F32)
        nc.vector.tensor_copy(scale2[:], sc2_ps[:])

        # x hi/lo bf16 split, interleaved as xs[:, c, 0]=xh, xs[:, c, 1]=xl
        xh_bf = const.tile([128, feat_chunks], BF16)
        nc.vector.tensor_copy(xh_bf[:], xt[:])
        xh32 = const.tile([128, feat_chunks], F32)
        nc.vector.tensor_copy(xh32[:], xh_bf[:])
        xl32 = const.tile([128, feat_chunks], F32)
        nc.vector.tensor_sub(xl32[:], xt[:], xh32[:])
        xs = const.tile([128, feat_chunks, 2], BF16)
        nc.vector.tensor_copy(xs[:, :, 0:1], xh_bf[:].rearrange("p (c o) -> p c o", o=1))
        nc.vector.tensor_copy(xs[:, :, 1:2], xl32[:].rearrange("p (c o) -> p c o", o=1))

        # ---- persistent [2, nc_half] accumulators (row0: hi-part, row1: lo-x part)
        pp_a = psum_p.tile([2, nc_half], F32, tag="pp_a")
        pp_b = psum_p.tile([2, nc_half], F32, tag="pp_b")

        def emit_post(ps_s, vh, w):
            # fused: ps_t[128,1] = scale*(row0+row1) transposed, per 128-chunk
            s2 = s_pool.tile([2, win], F32, tag="s2")
            nc.vector.tensor_copy(s2[:], ps_s[:])
            ss = st_pool.tile([128, chunks_per_win, 2], BF16)
            for q in range(chunks_per_win):
                ps_t = psum_t.tile([128, 1], F32, tag="ps_t")
                nc.tensor.matmul(
                    ps_t[:],
                    lhsT=s2[:, q * 128:(q + 1) * 128],
                    rhs=scale2[:],
                    start=True,
                    stop=True,
                )
                se = st_pool.tile([128, 1], F32, tag="se")
                nc.scalar.activation(se[:], ps_t[:], AF.Exp)
                nc.vector.tensor_copy(ss[:, q, 0:1], se[:])
                sh32 = st_pool.tile([128, 1], F32, tag="sh32")
                nc.vector.tensor_copy(sh32[:], ss[:, q, 0:1])
                sl32 = st_pool.tile([128, 1], F32, tag="sl32")
                nc.vector.tensor_sub(sl32[:], se[:], sh32[:])
                nc.vector.tensor_copy(ss[:, q, 1:2], sl32[:])
            # stage 2: pp[0,:] += sh@V ; pp[1,:] += sl@V   (V is fp16)
            for q in range(chunks_per_win):
                gc = w * chunks_per_win + q
                first = gc == 0
                last = gc == n_chunks - 1
                for pp, j0 in ((pp_a, 0), (pp_b, nc_half)):
                    nc.tensor.matmul(
                        pp[:],
                        lhsT=ss[:, q, :],
                        rhs=vh[:, q, j0:j0 + nc_half],
                        start=first,
                        stop=last,
                        skip_group_check=True,
                    )

        # Software-pipelined emission, depth 2: window w's post-chain
        # (rowsum/exp/transpose/stage-2) is emitted after window w+2's
        # stage-1 matmuls, giving the ACT/DVE exp+cast chain a full extra
        # window to complete before the PE needs its stage-2 operands.
        pends = []
        for w in range(n_windows):
            kh = keys_pool.tile([128, feat_chunks, win], F16, tag="keys")
            nc.sync.dma_start(
                out=kh[:], in_=kh_d[w].rearrange("p (c j) -> p c j", c=feat_chunks)
            )
            vh = vals_pool.tile([128, chunks_per_win, n_classes], F16, tag="vals")
            nc.sync.dma_start(
                out=vh[:], in_=vh_d[w].rearrange("p (q j) -> p q j", q=chunks_per_win)
            )

            # stage 1: ps_s[0,:] = xh@K ; ps_s[1,:] = xl@K   (K is fp16)
            ps_s = psum_s.tile([2, win], F32)
            for c in range(feat_chunks):
                nc.tensor.matmul(
                    ps_s[:],
                    lhsT=xs[:, c, :],
                    rhs=kh[:, c, :],
                    start=(c == 0),
                    stop=(c == feat_chunks - 1),
                    skip_group_check=True,
                )

            pends.append((ps_s, vh, w))
            if len(pends) > 2:
                emit_post(*pends.pop(0))
        for p in pends:
            emit_post(*p)

        # ---- tail: p = row0 + row1 (copy to SBUF, K=2 ones-matmul row sum)
        p_sb = const.tile([1, n_classes], F32)
        for pp, j0 in ((pp_a, 0), (pp_b, nc_half)):
            pc = const.tile([2, nc_half], F32, tag=f"pc{j0}")
            nc.vector.tensor_copy(pc[:], pp[:])
            pr = psum_t.tile([1, nc_half], F32, tag="ps_t")
            nc.tensor.matmul(
                pr[:], lhsT=ones[0:2, 0:1], rhs=pc[:], start=True, stop=True
            )
            nc.vector.tensor_copy(p_sb[:, j0:j0 + nc_half], pr[:])

        partial = dram.tile([1, n_classes], F32)
        reduced = dram.tile([1, n_classes], F32)
        nc.gpsimd.dma_start(partial[:], p_sb[:])
        nc.gpsimd.collective_compute(
            "AllReduce",
            mybir.AluOpType.add,
            replica_groups=[list(range(num_devices))],
            ins=[partial.opt()],
            outs=[reduced.opt()],
        )
        red_sb = const.tile([1, n_classes], F32)
        nc.sync.dma_start(red_sb[:], reduced[:])
        logp = const.tile([1, n_classes], F32)
        nc.scalar.activation(logp[:], red_sb[:], AF.Ln)
        nc.sync.dma_start(out_d[:], logp[:])

    nc.compile()
    return nc


_NC_CACHE: dict = {}


def _get_nc():
    if "nc" not in _NC_CACHE:
        _NC_CACHE["nc"] = build_kernel()
    return _NC_CACHE["nc"]


def _split_hi_lo(a):
    hi = a.astype(BF16_NP)
    lo = (a - hi.astype(np.float32)).astype(BF16_NP)
    return hi, lo


def _retile_keys(keys_shard, feat_chunks=FEAT_CHUNKS, win=WIN):
    """[d_feat, n_pad] -> [n_windows, 128, feat_chunks*win] with
    out[w, p, c*win + j] = keys_shard[c*128 + p, w*win + j]."""
    d_feat, n_pad = keys_shard.shape
    n_windows = n_pad // win
    v = keys_shard.reshape(feat_chunks, 128, n_windows, win)
    return np.ascontiguousarray(v.transpose(2, 1, 0, 3)).reshape(
        n_windows, 128, feat_chunks * win
    )


def _retile_vals(vals_shard, chunks_per_win=CHUNKS_PER_WIN, win=WIN):
    """[n_pad, n_classes] -> [n_windows, 128, chunks_per_win*n_classes] with
    out[w, p, q*ncls + j] = vals_shard[(w*cpw + q)*128 + p, j]."""
    n_pad, ncls = vals_shard.shape
    n_windows = n_pad // win
    v = vals_shard.reshape(n_windows, chunks_per_win, 128, ncls)
    return np.ascontiguousarray(v.transpose(0, 2, 1, 3)).reshape(
        n_windows, 128, chunks_per_win * ncls
    )


def _shard_inputs(x, mem_keys, mem_vals):
    x = np.ascontiguousarray(np.asarray(x, dtype=np.float32))
    in_maps = []
    for i in range(N_CORES):
        lo_i, hi_i = i * N_SHARD, (i + 1) * N_SHARD
        keys_shard = np.zeros((D_FEAT, N_PAD), dtype=np.float32)
        keys_shard[:, :N_SHARD] = mem_keys[:, lo_i:hi_i]
        vals_shard = np.zeros((N_PAD, N_CLASSES), dtype=np.float32)
        vals_shard[:N_SHARD, :] = mem_vals[lo_i:hi_i, :]
        kh = _retile_keys(keys_shard).astype(np.float16)
        vh = _retile_vals(vals_shard).astype(np.float16)
        in_maps.append({"x": x, "kh": kh, "vh": vh})
    return in_maps


def run(x, mem_keys, mem_vals, trace: bool = False):
    """Runs the SPMD kernel; returns (output [1, N_CLASSES], BassKernelResults)."""
    from concourse.bass_utils import run_bass_kernel_spmd

    nc = _get_nc()
    in_maps = _shard_inputs(x, mem_keys, mem_vals)
    res = run_bass_kernel_spmd(nc, in_maps, list(range(N_CORES)), trace=trace)
    out = np.asarray(res.results[0]["out"], dtype=np.float32).reshape(1, N_CLASSES)
    return out, res


def kernel(x, mem_keys, mem_vals):
    out, _ = run(x, mem_keys, mem_vals, trace=False)
    return out
